# revision 1
# baseline (speedup 1.0000x reference)
"""Trainium2 Bass kernel for DFine multi-head attention.

Problem: B=2, S=2048, D=1024, H=16 heads, HD=64.
Sharding over 8 cores: core c handles batch b=c//4 and head-group g=c%4
(4 heads). Each core computes its heads' attention and a partial
out-projection [2048, 1024]; the host sums the 4 partials per batch and
adds the output bias.

All matmuls run in float32r (TF32-like, full PE rate for moving dim
>= 256, ~1.5e-4 relative error).
"""

import sys
import numpy as np

if "/opt/trn_rl_repo" not in sys.path:
    sys.path.insert(0, "/opt/trn_rl_repo")

B, S, D, H, HD = 2, 2048, 1024, 16, 64
G = 4          # heads per core
E = G * HD     # 256 per-core head width
T = S          # tokens
KC = 8         # contraction chunks of 128 over D
TB = 512       # t-block (moving free dim)
NT = T // TB   # 4
NS = T // 128  # 16 s-chunks
SCALE = HD ** -0.5

_PROGRAM = None


def _build_program():
    import concourse.bacc as bacc
    import concourse.tile as tile
    from concourse import mybir

    fr = mybir.dt.float32r
    f32 = mybir.dt.float32
    Exp = mybir.ActivationFunctionType.Exp

    nc = bacc.Bacc("TRN2", target_bir_lowering=False, debug=False)

    xT_d = nc.declare_dram_parameter("xT", [D, T], f32, isOutput=False)
    pT_d = nc.declare_dram_parameter("pT", [D, T], f32, isOutput=False)
    wq_d = nc.declare_dram_parameter("wq", [D, E], f32, isOutput=False)
    wk_d = nc.declare_dram_parameter("wk", [D, E], f32, isOutput=False)
    wv_d = nc.declare_dram_parameter("wv", [D, E], f32, isOutput=False)
    wo_d = nc.declare_dram_parameter("wo", [E, D], f32, isOutput=False)
    bq_d = nc.declare_dram_parameter("bq", [2, 128, 1], f32, isOutput=False)
    bk_d = nc.declare_dram_parameter("bk", [2, 128, 1], f32, isOutput=False)
    bv_d = nc.declare_dram_parameter("bvr", [128, E], f32, isOutput=False)
    out_d = nc.declare_dram_parameter("out", [T, D], f32, isOutput=True)

    with tile.TileContext(nc) as tc:
        from contextlib import ExitStack

        octx = ExitStack()
        wpool = octx.enter_context(tc.tile_pool(name="wpool", bufs=1))
        qkpool = octx.enter_context(tc.tile_pool(name="qkpool", bufs=1))
        vpool = octx.enter_context(tc.tile_pool(name="vpool", bufs=1))
        a2pool = octx.enter_context(tc.tile_pool(name="a2pool", bufs=1))

        # ---- persistent tiles ----
        wq_t = wpool.tile([128, KC, E], fr, name="wq_t")
        wk_t = wpool.tile([128, KC, E], fr, name="wk_t")
        wv_t = wpool.tile([128, KC, E], fr, name="wv_t")
        wo_t = wpool.tile([128, 2, D], fr, name="wo_t")
        bq_t = wpool.tile([128, 2, 1], f32, name="bq_t")
        bk_t = wpool.tile([128, 2, 1], f32, name="bk_t")
        bv_t = wpool.tile([128, E], f32, name="bv_t")
        ones_f = wpool.tile([1, 64], f32, name="ones_f")
        ones_r = wpool.tile([1, 64], fr, name="ones_r")
        oneblk = wpool.tile([128, NS, G, 1], f32, name="oneblk")

        qT = [qkpool.tile([128, T], fr, name=f"qT{p}") for p in range(2)]
        kT = [qkpool.tile([128, T], fr, name=f"kT{p}") for p in range(2)]
        v_aug = vpool.tile([128, NS, G, HD + 1], fr, name="v_aug")
        at2 = [a2pool.tile([128, T], fr, name=f"at2_{p}") for p in range(2)]

        # ---- weight / bias DMAs ----
        nc.sync.dma_start(
            wq_t[:], wq_d[:].bitcast(fr).rearrange("(c p) e -> p c e", p=128))
        nc.sync.dma_start(
            wk_t[:], wk_d[:].bitcast(fr).rearrange("(c p) e -> p c e", p=128))
        nc.sync.dma_start(
            wv_t[:], wv_d[:].bitcast(fr).rearrange("(c p) e -> p c e", p=128))
        nc.sync.dma_start(
            wo_t[:], wo_d[:].bitcast(fr).rearrange("(c p) d -> p c d", p=128))
        nc.sync.dma_start(bq_t[:], bq_d[:].rearrange("c p o -> p c o"))
        nc.sync.dma_start(bk_t[:], bk_d[:].rearrange("c p o -> p c o"))
        nc.sync.dma_start(bv_t[:], bv_d[:])
        nc.vector.memset(ones_f[:], 1.0)
        nc.vector.tensor_copy(ones_r[:], ones_f[:])
        nc.vector.memset(oneblk[:], 1.0)
        # ones column of v_aug
        nc.vector.tensor_copy(v_aug[:, :, :, HD:HD + 1], oneblk[:])

        # ---- phase A/B: projections ----
        with tc.tile_pool(name="xpool", bufs=1) as xpool, \
             tc.tile_pool(name="ppool", bufs=2) as ppool, \
             tc.tile_pool(name="vps", bufs=2, space="PSUM") as vps, \
             tc.tile_pool(name="qkps", bufs=3, space="PSUM") as qkps:
            xT_t = xpool.tile([128, KC, T], fr, name="xT_t")
            nc.sync.dma_start(
                xT_t[:], xT_d[:].bitcast(fr).rearrange("(c p) t -> p c t", p=128))

            # v projection, token-major: v[s, e] accumulated over d-chunks
            for si in range(NS):
                ps = vps.tile([128, E], f32, name=f"vps{si}", tag="vps")
                for k in range(KC):
                    nc.tensor.matmul(
                        ps[:], xT_t[:, k, si * 128:(si + 1) * 128],
                        wv_t[:, k, :], start=(k == 0), stop=(k == KC - 1))
                nc.vector.tensor_tensor(
                    v_aug[:, si, :, 0:HD],
                    ps[:].rearrange("p (g e) -> p g e", g=G),
                    bv_t[:].rearrange("p (g e) -> p g e", g=G),
                    op=mybir.AluOpType.add)

            # hT = xT + pT, in place (after v-proj consumed xT chunk)
            for k in range(KC):
                pt = ppool.tile([128, T], fr, name=f"pt{k}", tag="pt")
                nc.sync.dma_start(
                    pt[:], pT_d[:].bitcast(fr)[k * 128:(k + 1) * 128, :])
                nc.vector.tensor_tensor(
                    xT_t[:, k, :], xT_t[:, k, :], pt[:],
                    op=mybir.AluOpType.add)

            # q/k projections into head-pair-major transposed layout
            for w_t, b_t, dsts in ((wq_t, bq_t, qT), (wk_t, bk_t, kT)):
                for p in range(2):
                    for tb in range(NT):
                        ps = qkps.tile([128, TB], f32, name=f"qk{p}{tb}",
                                       tag="qkps")
                        for k in range(KC):
                            nc.tensor.matmul(
                                ps[:],
                                w_t[:, k, p * 128:(p + 1) * 128],
                                xT_t[:, k, tb * TB:(tb + 1) * TB],
                                start=(k == 0), stop=(k == KC - 1))
                        nc.vector.tensor_scalar_add(
                            dsts[p][:, tb * TB:(tb + 1) * TB], ps[:],
                            b_t[:, p, :])

        # ---- phase C/D: attention + out-projection ----
        epool = octx.enter_context(tc.tile_pool(name="epool", bufs=4))
        npool = octx.enter_context(tc.tile_pool(name="npool", bufs=2))
        opool = octx.enter_context(tc.tile_pool(name="opool", bufs=2))
        cps = octx.enter_context(tc.tile_pool(name="cps", bufs=1, space="PSUM"))

        for tb in range(NT):
            t0 = tb * TB
            for p in range(2):
                atp = [cps.tile([HD + 1, TB], f32, name=f"at_{tb}_{p}_{h}",
                                tag=f"at{h}") for h in range(2)]
                for sg in range(NS // 2):
                    scp = cps.tile([128, 2, 2, TB], f32,
                                   name=f"sc_{tb}_{p}_{sg}", tag="sc")
                    for s2 in range(2):
                        si = sg * 2 + s2
                        for h in range(2):
                            nc.tensor.matmul(
                                scp[:, s2, h, :],
                                kT[p][h * 64:(h + 1) * 64,
                                      si * 128:(si + 1) * 128],
                                qT[p][h * 64:(h + 1) * 64, t0:t0 + TB],
                                start=True, stop=True)
                    ex = epool.tile([128, 2, 2, TB], fr,
                                    name=f"ex_{tb}_{p}_{sg}", tag="exp")
                    nc.scalar.activation(ex[:], scp[:], Exp)
                    for s2 in range(2):
                        si = sg * 2 + s2
                        for h in range(2):
                            nc.tensor.matmul(
                                atp[h][:],
                                v_aug[:, si, p * 2 + h, :],
                                ex[:, s2, h, :],
                                start=(si == 0), stop=(si == NS - 1),
                                skip_group_check=True)
                # normalize heads of this pair
                for h in range(2):
                    rec = npool.tile([1, TB], fr, name=f"rc_{tb}_{p}_{h}",
                                     tag="rec")
                    with nc.allow_low_precision(reason="f32r recip"):
                        nc.vector.reciprocal(rec[:], atp[h][HD:HD + 1, :])
                    bc = cps.tile([64, TB], f32, name=f"bc_{tb}_{p}_{h}",
                                  tag="op")
                    nc.tensor.matmul(bc[:], ones_r[:], rec[:],
                                     start=True, stop=True)
                    scr = npool.tile([64, TB], f32, name=f"scr_{tb}_{p}_{h}",
                                     tag="scr")
                    nc.vector.tensor_copy(scr[:], atp[h][0:HD, :])
                    nc.vector.tensor_tensor(
                        at2[p][h * 64:(h + 1) * 64, t0:t0 + TB],
                        scr[:], bc[:], op=mybir.AluOpType.mult)

            # out-projection for this t-block
            for ts in range(TB // 128):
                tsl = t0 + ts * 128
                osb = opool.tile([128, D], f32, name=f"osb_{tb}_{ts}",
                                 tag="osb")
                for dc in range(2):
                    ps = cps.tile([128, 512], f32, name=f"op_{tb}_{ts}_{dc}",
                                  tag="op")
                    for p in range(2):
                        nc.tensor.matmul(
                            ps[:], at2[p][:, tsl:tsl + 128],
                            wo_t[:, p, dc * 512:(dc + 1) * 512],
                            start=(p == 0), stop=(p == 1))
                    nc.vector.tensor_copy(osb[:, dc * 512:(dc + 1) * 512],
                                          ps[:])
                nc.sync.dma_start(out_d[tsl:tsl + 128, :], osb[:])

        octx.close()

    nc.compile()
    return nc


def _get_program():
    global _PROGRAM
    if _PROGRAM is None:
        _PROGRAM = _build_program()
    return _PROGRAM


def _shard_inputs(inputs):
    """Build the 8 per-core input maps from the full-problem inputs."""
    hs = np.asarray(inputs["hidden_states"], np.float32)
    pe = np.asarray(inputs["position_embeddings"], np.float32)
    Wq = np.asarray(inputs["Wq"], np.float32).reshape(D, H * HD)
    Wk = np.asarray(inputs["Wk"], np.float32).reshape(D, H * HD)
    Wv = np.asarray(inputs["Wv"], np.float32).reshape(D, H * HD)
    Wo = np.asarray(inputs["Wo"], np.float32)
    bq = np.asarray(inputs["bq"], np.float32).reshape(H * HD)
    bk = np.asarray(inputs["bk"], np.float32).reshape(H * HD)
    bv = np.asarray(inputs["bv"], np.float32).reshape(H * HD)

    xT = [np.ascontiguousarray(hs[b].T) for b in range(B)]
    pT = [np.ascontiguousarray(pe[b].T) for b in range(B)]

    in_maps = []
    for c in range(8):
        b, g = divmod(c, G)
        sel = slice(g * E, (g + 1) * E)
        in_maps.append({
            "xT": xT[b],
            "pT": pT[b],
            "wq": np.ascontiguousarray(Wq[:, sel]) * np.float32(SCALE),
            "wk": np.ascontiguousarray(Wk[:, sel]),
            "wv": np.ascontiguousarray(Wv[:, sel]),
            "wo": np.ascontiguousarray(Wo[sel, :]),
            "bq": (bq[sel] * np.float32(SCALE)).reshape(2, 128, 1).copy(),
            "bk": bk[sel].reshape(2, 128, 1).copy(),
            "bvr": np.tile(bv[sel][None, :], (128, 1)),
        })
    return in_maps


def _gather_outputs(results, inputs):
    bo = np.asarray(inputs["bo"], np.float32)
    out = np.empty((B, S, D), np.float32)
    for b in range(B):
        acc = results[4 * b]["out"].astype(np.float32).copy()
        for g in range(1, G):
            acc += results[4 * b + g]["out"]
        out[b] = acc + bo[None, :]
    return out


def kernel(**inputs):
    from concourse.bass_utils import run_bass_kernel_spmd

    nc = _get_program()
    in_maps = _shard_inputs(inputs)
    res = run_bass_kernel_spmd(nc, in_maps, list(range(8)))
    return _gather_outputs(res.results, inputs)


# revision 11
# speedup vs baseline: 383.3107x; 383.3107x over previous
"""Trainium2 Bass kernel for DFine multi-head attention.

Problem: B=2, S=2048, D=1024, H=16 heads, HD=64.
Sharding over 8 cores: core c handles batch b=c//4 and head-group g=c%4
(4 heads). Each core computes its heads' attention and a partial
out-projection [2048, 1024]; the host sums the 4 partials per batch and
adds the output bias.

All matmuls run in float32r (TF32-like, full PE rate for moving dim
>= 256, ~1.5e-4 relative error).
"""

import sys
import numpy as np

if "/opt/trn_rl_repo" not in sys.path:
    sys.path.insert(0, "/opt/trn_rl_repo")

B, S, D, H, HD = 2, 2048, 1024, 16, 64
G = 4          # heads per core
E = G * HD     # 256 per-core head width
T = S          # tokens
KC = 8         # contraction chunks of 128 over D
TB = 512       # t-block (moving free dim)
NT = T // TB   # 4
NS = T // 128  # 16 s-chunks
SCALE = HD ** -0.5

_PROGRAM = None


def _build_program(reps=1):
    import concourse.bacc as bacc
    import concourse.tile as tile
    from concourse import mybir

    f32 = mybir.dt.float32

    nc = bacc.Bacc("TRN2", target_bir_lowering=False, debug=False)

    xT_d = nc.declare_dram_parameter("xT", [D, T], f32, isOutput=False)
    pT_d = nc.declare_dram_parameter("pT", [D, T], f32, isOutput=False)
    wq_d = nc.declare_dram_parameter("wq", [D, E], f32, isOutput=False)
    wk_d = nc.declare_dram_parameter("wk", [D, E], f32, isOutput=False)
    wv_d = nc.declare_dram_parameter("wv", [D, E], f32, isOutput=False)
    wo_d = nc.declare_dram_parameter("wo", [E, D], f32, isOutput=False)
    bq_d = nc.declare_dram_parameter("bq", [2, 128, 1], f32, isOutput=False)
    bk_d = nc.declare_dram_parameter("bk", [2, 128, 1], f32, isOutput=False)
    bv_d = nc.declare_dram_parameter("bvr", [128, E], f32, isOutput=False)
    out_d = nc.declare_dram_parameter("out", [T, D], f32, isOutput=True)

    with tile.TileContext(nc) as tc:
        for rep in range(reps):
            _build_body(nc, tc, mybir, rep,
                        (xT_d, pT_d, wq_d, wk_d, wv_d, wo_d, bq_d, bk_d,
                         bv_d, out_d))

    nc.compile()
    return nc


def _build_body(nc, tc, mybir, rep, drams):
    from contextlib import ExitStack

    fr = mybir.dt.float32r
    f32 = mybir.dt.float32
    Exp = mybir.ActivationFunctionType.Exp
    (xT_d, pT_d, wq_d, wk_d, wv_d, wo_d, bq_d, bk_d, bv_d, out_d) = drams
    R = f"r{rep}_"

    octx = ExitStack()
    wpool = octx.enter_context(tc.tile_pool(name=f"{R}wpool", bufs=1))
    qkpool = octx.enter_context(tc.tile_pool(name=f"{R}qkpool", bufs=1))
    vpool = octx.enter_context(tc.tile_pool(name=f"{R}vpool", bufs=1))

    # ---- persistent tiles ----
    wq_t = wpool.tile([128, KC, E], fr, name=f"{R}wq_t")
    wk_t = wpool.tile([128, KC, E], fr, name=f"{R}wk_t")
    wv_t = wpool.tile([128, KC, E], fr, name=f"{R}wv_t")
    bq_t = wpool.tile([128, 2, 1], f32, name=f"{R}bq_t")
    bk_t = wpool.tile([128, 2, 1], f32, name=f"{R}bk_t")
    bv_t = wpool.tile([128, E], f32, name=f"{R}bv_t")
    ones_f = wpool.tile([1, 64], f32, name=f"{R}ones_f")
    ones_r = wpool.tile([1, 64], fr, name=f"{R}ones_r")
    oneblk = wpool.tile([128, NS, G, 1], f32, name=f"{R}oneblk")

    qT = [qkpool.tile([128, T], fr, name=f"{R}qT{p}") for p in range(2)]
    kT = [qkpool.tile([128, T], fr, name=f"{R}kT{p}") for p in range(2)]
    v_aug = vpool.tile([128, NS, G, HD + 1], fr, name=f"{R}v_aug")

    # ---- weight / bias DMAs (first: v-proj needs wv immediately) ----
    nc.gpsimd.dma_start(
        wv_t[:], wv_d[:].bitcast(fr).rearrange("(c p) e -> p c e", p=128))
    nc.gpsimd.dma_start(
        wq_t[:], wq_d[:].bitcast(fr).rearrange("(c p) e -> p c e", p=128))
    nc.gpsimd.dma_start(
        wk_t[:], wk_d[:].bitcast(fr).rearrange("(c p) e -> p c e", p=128))
    nc.gpsimd.dma_start(bq_t[:], bq_d[:].rearrange("c p o -> p c o"))
    nc.gpsimd.dma_start(bk_t[:], bk_d[:].rearrange("c p o -> p c o"))
    nc.gpsimd.dma_start(bv_t[:], bv_d[:])
    nc.vector.memset(ones_f[:], 1.0)
    nc.vector.tensor_copy(ones_r[:], ones_f[:])
    nc.vector.memset(oneblk[:], 1.0)
    nc.vector.tensor_copy(v_aug[:, :, :, HD:HD + 1], oneblk[:])

    # ---- phase A/B: projections (DMA-overlapped, k-outer) ----
    ictx = ExitStack()
    ppool = ictx.enter_context(tc.tile_pool(name=f"{R}ppool", bufs=1))
    hT_t = ppool.tile([128, KC, T], fr, name=f"{R}hT_t")
    qkps = ictx.enter_context(tc.tile_pool(name=f"{R}qkps", bufs=1,
                                           space="PSUM"))

    actx = ExitStack()
    xpool = actx.enter_context(tc.tile_pool(name=f"{R}xpool", bufs=1))
    qkpsB = actx.enter_context(tc.tile_pool(name=f"{R}qkpsB", bufs=1,
                                            space="PSUM"))
    vps = actx.enter_context(tc.tile_pool(name=f"{R}vps", bufs=4,
                                          space="PSUM"))
    xT_t = xpool.tile([128, KC, T], fr, name=f"{R}xT_t")
    for k in range(KC):
        nc.sync.dma_start(
            xT_t[:, k, :], xT_d[:].bitcast(fr)[k * 128:(k + 1) * 128, :])
        nc.sync.dma_start(
            hT_t[:, k, :], pT_d[:].bitcast(fr)[k * 128:(k + 1) * 128, :])

    # hT = xT + pT in place on the pT tiles (gated only by the two DMAs)
    for k in range(KC):
        nc.vector.tensor_tensor(
            hT_t[:, k, :], hT_t[:, k, :], xT_t[:, k, :],
            op=mybir.AluOpType.add)

    # v projection, token-major: two 4-chunk windows accumulated into
    # v_aug so psum slots stay short-lived (4 banks, tag-cycled)
    for w in range(2):
        for si in range(NS):
            ps = vps.tile([128, E], f32, name=f"{R}vp{w}_{si}", tag="v")
            for kk in range(4):
                k = w * 4 + kk
                nc.tensor.matmul(
                    ps[:], xT_t[:, k, si * 128:(si + 1) * 128],
                    wv_t[:, k, :], start=(kk == 0), stop=(kk == 3))
            dst = v_aug[:, si, :, 0:HD]
            psg = ps[:].rearrange("p (g e) -> p g e", g=G)
            if w == 0:
                nc.vector.tensor_tensor(
                    dst, psg, bv_t[:].rearrange("p (g e) -> p g e", g=G),
                    op=mybir.AluOpType.add)
            else:
                nc.vector.tensor_tensor(dst, dst, psg,
                                        op=mybir.AluOpType.add)

    # q/k projections: 8 k-outer waves of 2 psum groups; pair-0 waves
    # emitted now, pair-1 waves are emitted mid-phase-C to fill PE idle
    def qk_wave(w_t, b_t, dsts, nm, p, tbs, wide=False):
        pss = {}
        for tb in tbs:
            pool = qkpsB if (wide and tb >= 2) else qkps
            pss[tb] = pool.tile([128, TB], f32, name=f"{R}{nm}ps{p}{tb}",
                                tag=f"qk{tb % 2}")
        for k in range(KC):
            for tb in tbs:
                nc.tensor.matmul(
                    pss[tb][:],
                    w_t[:, k, p * 128:(p + 1) * 128],
                    hT_t[:, k, tb * TB:(tb + 1) * TB],
                    start=(k == 0), stop=(k == KC - 1))
        for tb in tbs:
            nc.vector.tensor_scalar_add(
                dsts[p][:, tb * TB:(tb + 1) * TB], pss[tb][:], b_t[:, p, :])

    for w_t, b_t, dsts, nm in ((wq_t, bq_t, qT, "q"), (wk_t, bk_t, kT, "k")):
        qk_wave(w_t, b_t, dsts, nm, 0, (0, 1, 2, 3), wide=True)

    actx.close()  # frees xT + v psum + wide qk psum

    # ---- phase C/D: attention + out-projection ----
    cctx = ExitStack()
    a2pool = cctx.enter_context(tc.tile_pool(name=f"{R}a2pool", bufs=1))
    epool = cctx.enter_context(tc.tile_pool(name=f"{R}epool", bufs=6))
    npool = cctx.enter_context(tc.tile_pool(name=f"{R}npool", bufs=2))
    opool = cctx.enter_context(tc.tile_pool(name=f"{R}opool", bufs=2))
    scps = cctx.enter_context(tc.tile_pool(name=f"{R}scps", bufs=2,
                                           space="PSUM"))
    atps = cctx.enter_context(tc.tile_pool(name=f"{R}atps", bufs=1,
                                           space="PSUM"))

    at2 = [a2pool.tile([128, T], fr, name=f"{R}at2_{p}") for p in range(2)]
    wo_t = a2pool.tile([128, 2, D], fr, name=f"{R}wo_t")
    nc.gpsimd.dma_start(
        wo_t[:], wo_d[:].bitcast(fr).rearrange("(c p) d -> p c d", p=128))

    def attention_pair(tb, p):
        t0 = tb * TB
        atp = [atps.tile([HD + 1, TB], f32, name=f"{R}at_{tb}_{p}_{h}",
                         tag=f"at{h}") for h in range(2)]
        for si in range(NS):
            scp = scps.tile([128, 2, TB], f32,
                            name=f"{R}sc_{tb}_{p}_{si}", tag="sc")
            for h in range(2):
                nc.tensor.matmul(
                    scp[:, h, :],
                    kT[p][h * 64:(h + 1) * 64, si * 128:(si + 1) * 128],
                    qT[p][h * 64:(h + 1) * 64, t0:t0 + TB],
                    start=True, stop=True)
            ex = epool.tile([128, 2, TB], fr,
                            name=f"{R}ex_{tb}_{p}_{si}", tag="exp")
            nc.scalar.activation(ex[:], scp[:], Exp)
            for h in range(2):
                nc.tensor.matmul(
                    atp[h][:],
                    v_aug[:, si, p * 2 + h, :],
                    ex[:, h, :],
                    start=(si == 0), stop=(si == NS - 1),
                    skip_group_check=True)
        # normalize heads of this pair; bc reuses the freed at-slot
        for h in range(2):
            rec = npool.tile([1, TB], fr, name=f"{R}rc_{tb}_{p}_{h}",
                             tag="rec")
            with nc.allow_low_precision(reason="f32r recip"):
                nc.vector.reciprocal(rec[:], atp[h][HD:HD + 1, :])
            scr = npool.tile([64, TB], f32, name=f"{R}scr_{tb}_{p}_{h}",
                             tag="scr")
            nc.vector.tensor_copy(scr[:], atp[h][0:HD, :])
            bc = atps.tile([64, TB], f32, name=f"{R}bc_{tb}_{p}_{h}",
                           tag=f"at{h}")
            nc.tensor.matmul(bc[:], ones_r[:], rec[:], start=True, stop=True)
            nc.vector.tensor_tensor(
                at2[p][h * 64:(h + 1) * 64, t0:t0 + TB],
                scr[:], bc[:], op=mybir.AluOpType.mult)

    def out_proj(tb):
        t0 = tb * TB
        for ts in range(TB // 128):
            tsl = t0 + ts * 128
            osb = opool.tile([128, D], f32, name=f"{R}osb_{tb}_{ts}",
                             tag="osb")
            for dc in range(2):
                ps = qkps.tile([128, 512], f32, name=f"{R}op_{tb}_{ts}_{dc}",
                               tag=f"qk{dc}")
                for p in range(2):
                    nc.tensor.matmul(
                        ps[:], at2[p][:, tsl:tsl + 128],
                        wo_t[:, p, dc * 512:(dc + 1) * 512],
                        start=(p == 0), stop=(p == 1))
                nc.vector.tensor_copy(osb[:, dc * 512:(dc + 1) * 512], ps[:])
            nc.sync.dma_start(out_d[tsl:tsl + 128, :], osb[:])

    attention_pair(0, 0)
    # pair-1 projections: low priority, fill PE idle under phase C's ACT
    for w_t, b_t, dsts, nm in ((wq_t, bq_t, qT, "q"), (wk_t, bk_t, kT, "k")):
        qk_wave(w_t, b_t, dsts, nm, 1, (0, 1))
        qk_wave(w_t, b_t, dsts, nm, 1, (2, 3))
    attention_pair(0, 1)
    for tb in range(1, NT):
        attention_pair(tb, 0)
        out_proj(tb - 1)
        attention_pair(tb, 1)
    out_proj(NT - 1)

    cctx.close()
    ictx.close()  # frees hT + qk psum
    octx.close()



def _get_program(reps=1):
    global _PROGRAM
    if _PROGRAM is None:
        _PROGRAM = {}
    if reps not in _PROGRAM:
        _PROGRAM[reps] = _build_program(reps)
    return _PROGRAM[reps]


def _shard_inputs(inputs):
    """Build the 8 per-core input maps from the full-problem inputs."""
    hs = np.asarray(inputs["hidden_states"], np.float32)
    pe = np.asarray(inputs["position_embeddings"], np.float32)
    Wq = np.asarray(inputs["Wq"], np.float32).reshape(D, H * HD)
    Wk = np.asarray(inputs["Wk"], np.float32).reshape(D, H * HD)
    Wv = np.asarray(inputs["Wv"], np.float32).reshape(D, H * HD)
    Wo = np.asarray(inputs["Wo"], np.float32)
    bq = np.asarray(inputs["bq"], np.float32).reshape(H * HD)
    bk = np.asarray(inputs["bk"], np.float32).reshape(H * HD)
    bv = np.asarray(inputs["bv"], np.float32).reshape(H * HD)

    xT = [np.ascontiguousarray(hs[b].T) for b in range(B)]
    pT = [np.ascontiguousarray(pe[b].T) for b in range(B)]

    in_maps = []
    for c in range(8):
        b, g = divmod(c, G)
        sel = slice(g * E, (g + 1) * E)
        in_maps.append({
            "xT": xT[b],
            "pT": pT[b],
            "wq": np.ascontiguousarray(Wq[:, sel]) * np.float32(SCALE),
            "wk": np.ascontiguousarray(Wk[:, sel]),
            "wv": np.ascontiguousarray(Wv[:, sel]),
            "wo": np.ascontiguousarray(Wo[sel, :]),
            "bq": (bq[sel] * np.float32(SCALE)).reshape(2, 128, 1).copy(),
            "bk": bk[sel].reshape(2, 128, 1).copy(),
            "bvr": np.tile(bv[sel][None, :], (128, 1)),
        })
    return in_maps


def _gather_outputs(results, inputs):
    bo = np.asarray(inputs["bo"], np.float32)
    out = np.empty((B, S, D), np.float32)
    for b in range(B):
        acc = results[4 * b]["out"].astype(np.float32).copy()
        for g in range(1, G):
            acc += results[4 * b + g]["out"]
        out[b] = acc + bo[None, :]
    return out


def kernel(**inputs):
    from concourse.bass_utils import run_bass_kernel_spmd

    nc = _get_program()
    in_maps = _shard_inputs(inputs)
    res = run_bass_kernel_spmd(nc, in_maps, list(range(8)))
    return _gather_outputs(res.results, inputs)
